# revision 1
# baseline (speedup 1.0000x reference)
"""Trainium2 kernel for the quantum-circuit AENN problem.

The reference applies a fixed 10-qubit variational circuit (186 params) to
each normalized input row, takes |amp|^2, rescales by norm^2, and applies a
Dense layer.  The circuit is LINEAR in the state, so it is a fixed 1024x1024
complex unitary U, and the normalization cancels exactly:

    norm^2 * |U (x/norm)|^2 = |U x|^2

so:  out = ((X @ Ur^T)^2 + (X @ Ui^T)^2) @ kernel + bias

Host side: build U from the 186 weights (tiny), quantize W = [Ur^T | Ui^T]
to fp8e3 (e3m4) with one scale per amplitude row (tied across Re/Im so the
scale squares out of |amp|^2 and folds into the host-side 1024x10 dense
layer), pre-transpose X to fp16.  Device side (pure data parallelism, batch
sharded 512 rows/core, no collectives): per amp-block pair t, Y^T =
W-block^T x X^T via TensorE (fp8e3 stationary x fp16 moving, fp32
accumulate -- measured exact to e3m4 precision), probs^T = Yr^2 + Yi^2
(ScalarE squares + VectorE add, bf16 out since the scaled probs overflow
fp16), DMA out.  fp8 weights halve the W DMA (4MB -> 2MB), which is what
bounds the critical prefix; all input DMAs ride one in-order HWDGE ring in
consumption order.  PE warm-up matmuls lift the HAM clock gate while the
input DMAs are in flight.
"""

import os
import numpy as np
import ml_dtypes

NUM_QUBITS = 10
LAYER_DEPTH = 4
DIM = 2 ** NUM_QUBITS            # 1024
BATCH = 4096
NUM_OUTPUT = 10
SIZE_ROT = (LAYER_DEPTH + 1) * NUM_QUBITS * 3   # 150
N_CORES = 8
ROWS = BATCH // N_CORES          # 512 rows per core
KT = DIM // 128                  # 8 k-tiles of 128 along the feature dim
AT = DIM // 128                  # 8 amplitude tile-pairs (Re,Im) of 128

_F16 = np.float16
_F8 = ml_dtypes.float8_e3m4
_F8_MAX = 15.5
_CACHE = {}
LAST_RESULTS = None  # BassKernelResults of the most recent run (for test.py)


# ----------------------------------------------------------------------------
# Host: build the circuit unitary U (amp = U @ psi)
# ----------------------------------------------------------------------------
def _build_unitary(qw: np.ndarray) -> np.ndarray:
    qw = np.asarray(qw, dtype=np.float64)
    rotations = qw[:SIZE_ROT].reshape(LAYER_DEPTH + 1, NUM_QUBITS, 3)
    rxx = qw[SIZE_ROT:].reshape(LAYER_DEPTH, NUM_QUBITS - 1)

    # Columns of the identity, qubit axes unpacked: shape (2,)*10 + (DIM,)
    M = np.eye(DIM, dtype=np.complex128).reshape((2,) * NUM_QUBITS + (DIM,))

    def apply_r(M, theta, phi, alpha, j):
        sa = np.sin(alpha)
        nx = sa * np.cos(phi)
        ny = sa * np.sin(phi)
        nz = np.cos(alpha)
        ct = np.cos(theta)
        mist = -1j * np.sin(theta)
        U2 = np.array([
            [ct + mist * nz, mist * (nx - 1j * ny)],
            [mist * (nx + 1j * ny), ct - mist * nz],
        ], dtype=np.complex128)
        M = np.tensordot(U2, M, axes=[[1], [j]])
        return np.moveaxis(M, 0, j)

    for k in range(LAYER_DEPTH):
        for j in range(NUM_QUBITS):
            M = apply_r(M, rotations[k, j, 0], rotations[k, j, 1],
                        rotations[k, j, 2], j)
        for j in range(NUM_QUBITS - 1):
            flipped = np.flip(M, axis=(j, j + 1))
            M = np.cos(rxx[k, j]) * M + (-1j * np.sin(rxx[k, j])) * flipped
    for j in range(NUM_QUBITS):
        M = apply_r(M, rotations[LAYER_DEPTH, j, 0],
                    rotations[LAYER_DEPTH, j, 1],
                    rotations[LAYER_DEPTH, j, 2], j)
    return M.reshape(DIM, DIM)   # U with amp = U @ psi


# ----------------------------------------------------------------------------
# Device graph (built once, cached)
# ----------------------------------------------------------------------------
# PE warm-up matmuls: lift the HAM clock gate AND keep PE busy until the
# first real matmul's inputs land (~9us) — any PE idle gap can re-throttle
# the clock gate depending on where the free-running HAM window lands.
# Cold N=512 matmuls issue at ~427ns each; warm-ups start ~8us and bridge
# to the first real matmul at ~10.6us (w slab0 + first xt quarter landed);
# pairs 0-1 are chunk-paced, so the few pre-HAM-flip matmuls cost little.
N_WARMUP = 8


def _build_graph():
    from concourse import bacc
    import concourse.mybir as mybir
    import concourse.tile as tile

    f16 = mybir.dt.float16
    f8 = mybir.dt.float8e3
    bf16 = mybir.dt.bfloat16

    nc = bacc.Bacc("TRN2", target_bir_lowering=False, debug=False,
                   num_devices=N_CORES)

    # xt[c, p, k', r] = X[r, 128*(2c+k')+p] (fp16) — four k-quarter DMAs of
    # 0.25MB each.  Small first chunks shorten the critical chain to the
    # first matmul (each DMA handoff costs transfer + ~0.8-1.8us completion
    # latency); pairs 0-1 then run chunk-major behind the arrivals.
    xt_d = nc.dram_tensor("xt", [KT // 2, 128, 2, ROWS], f16,
                          kind="ExternalInput")
    # w[t, p, k*256 + j] (fp8e3, row-scaled): j<128 -> s[128t+j]*Ur[128t+j,
    # 128k+p], j>=128 -> s[...]*Ui[...]
    w_d = nc.dram_tensor("w", [AT, 128, KT, 256], f8, kind="ExternalInput")
    # scaled probs^T tiles (bf16: values reach ~2e5, beyond fp16 range);
    # host applies the descaled 1024x10 dense layer + bias
    out_d = nc.dram_tensor("out", [AT, 128, ROWS], bf16, kind="ExternalOutput")

    with tile.TileContext(nc) as tc:
        with (
            tc.tile_pool(name="xtp", bufs=1) as xtp,
            tc.tile_pool(name="wp", bufs=AT) as wp,
            tc.tile_pool(name="cst", bufs=1) as cst,
            tc.tile_pool(name="sq", bufs=2) as sqp,
            tc.tile_pool(name="pb", bufs=2) as pbp,
            tc.tile_pool(name="psmm", bufs=3, space="PSUM") as psmm,
            tc.tile_pool(name="pswu", bufs=1, space="PSUM") as pswu,
        ):
            # PE warm-up on a zeroed scratch tile: no input deps, so these run
            # during the DMA wait and lift the HAM clock gate (PE 1.2 -> 2.4
            # GHz) right as the first real matmul's inputs land.  memset on
            # VectorE (idle in the preamble tail) so warm-ups start early.
            scratch = cst.tile([128, ROWS], f16)
            nc.vector.memset(scratch[:], 0.0)
            wu_ps = pswu.tile([128, ROWS], mybir.dt.float32)
            for _ in range(N_WARMUP):
                nc.tensor.matmul(wu_ps[:], scratch[:, 0:128], scratch[:],
                                 start=True, stop=True, skip_group_check=True)

            # Input DMAs ride BOTH in-order HWDGE rings in consumption
            # order: the sync ring carries the w slabs, the scalar ring
            # (qActDynamicHW) carries the xt k-quarters — so w0 and the
            # first xt chunk transfer concurrently from the very start and
            # the first matmul's operands land ~0.5us earlier than on a
            # single ring.  Within each ring, in-order processing keeps
            # later transfers from starving the critical prefix.
            w_slabs = [wp.tile([128, KT, 256], f8, name=f"wt{t}", tag="wt")
                       for t in range(AT)]
            xt_sb = xtp.tile([128, KT, ROWS], f16)
            for t in range(AT):
                nc.sync.dma_start(out=w_slabs[t][:], in_=w_d[t])
            for c in range(KT // 2):
                nc.scalar.dma_start(out=xt_sb[:, 2 * c:2 * c + 2, :],
                                    in_=xt_d[c])

            def wslab(t):
                return w_slabs[t][:]

            def epilogue(t, ps_re, ps_im, r0, nr):
                # psum tiles hold rows [r0, r0+nr) at column offset 0
                sq = sqp.tile([128, 2, ROWS], mybir.dt.float32, tag="sq")
                nc.scalar.square(sq[:, 0, 0:nr], ps_re[:, 0:nr])
                nc.scalar.square(sq[:, 1, 0:nr], ps_im[:, 0:nr])
                p_t = pbp.tile([128, ROWS], bf16, tag="p_t")
                nc.vector.tensor_add(p_t[:, 0:nr], sq[:, 0, 0:nr],
                                     sq[:, 1, 0:nr])
                nc.scalar.dma_start(out=out_d[t][:, r0:r0 + nr],
                                    in_=p_t[:, 0:nr])

            # Pairs 0-1 ride the four xt k-quarter arrivals: all four psum
            # groups stay open while the matmuls run chunk-major (k0-1 for
            # both pairs, then k2-3, ...), so the PE starts on the first
            # quarter and never waits longer than one chunk handoff.
            ps01 = {}
            for t in (0, 1):
                ps01[t] = (psmm.tile([128, ROWS], mybir.dt.float32,
                                     name=f"ps_re{t}", tag="ps_re"),
                           psmm.tile([128, ROWS], mybir.dt.float32,
                                     name=f"ps_im{t}", tag="ps_im"))
            for c in range(KT // 2):
                for t in (0, 1):
                    ps_re, ps_im = ps01[t]
                    wt = wslab(t)
                    for k in (2 * c, 2 * c + 1):
                        nc.tensor.matmul(ps_re[:], wt[:, k, 0:128],
                                         xt_sb[:, k, :],
                                         start=(k == 0), stop=(k == KT - 1))
                        nc.tensor.matmul(ps_im[:], wt[:, k, 128:256],
                                         xt_sb[:, k, :],
                                         start=(k == 0), stop=(k == KT - 1))
            for t in (0, 1):
                epilogue(t, ps01[t][0], ps01[t][1], 0, ROWS)

            def pair(t, wt, row_splits):
                """One amp-pair: 16 matmuls + epilogue per row-split; all re
                then all im so sq(re) overlaps the im matmuls."""
                for r0, nr in row_splits:
                    # fresh psum tiles per split: a shared tile would add a
                    # tile-granular WAR dependency on the previous split's
                    # epilogue reads
                    ps_re = psmm.tile([128, ROWS], mybir.dt.float32,
                                      tag="ps_re")
                    ps_im = psmm.tile([128, ROWS], mybir.dt.float32,
                                      tag="ps_im")
                    for k in range(KT):
                        nc.tensor.matmul(ps_re[:, 0:nr], wt[:, k, 0:128],
                                         xt_sb[:, k, r0:r0 + nr],
                                         start=(k == 0), stop=(k == KT - 1))
                    for k in range(KT):
                        nc.tensor.matmul(ps_im[:, 0:nr], wt[:, k, 128:256],
                                         xt_sb[:, k, r0:r0 + nr],
                                         start=(k == 0), stop=(k == KT - 1))
                    epilogue(t, ps_re, ps_im, r0, nr)

            for t in range(2, AT):
                if t < AT - 1:
                    pair(t, wslab(t), [(0, ROWS)])
                else:
                    # last pair: 384+128 row split so the final epilogue +
                    # out-DMA covers only 128 rows (short kernel tail)
                    pair(t, wslab(t), [(0, 384), (384, 128)])

    nc.compile()
    return nc


def _ensure_ntff_hook():
    """The trace path does `from antenv.axon_hooks import ...`; some images
    lack that optional module.  Provide it (wired to the axon PJRT .so when
    available) so BASS_TRACE=1 profiles instead of crashing."""
    try:
        import antenv.axon_hooks  # noqa: F401
        return
    except ImportError:
        pass
    import sys
    import types
    try:
        import antenv
    except ImportError:
        return
    mod = types.ModuleType("antenv.axon_hooks")
    state = {"hook": None}
    mod.set_axon_ntff_profile_hook = lambda h: state.__setitem__("hook", h)
    mod.get_axon_ntff_profile_hook = lambda: state["hook"]
    sys.modules["antenv.axon_hooks"] = mod
    antenv.axon_hooks = mod
    try:
        from trn_agent_boot.trn_boot import _ntff_profile_via_ctypes
        so_path = "/opt/axon/libaxon_pjrt.so"
        if os.path.exists(so_path):
            hook = _ntff_profile_via_ctypes(so_path)
            if hook is not None:
                mod.set_axon_ntff_profile_hook(hook)
    except Exception:
        pass


# ----------------------------------------------------------------------------
# Entry point
# ----------------------------------------------------------------------------
def kernel(x, quantum_weights, kernel, bias):
    global LAST_RESULTS
    _ensure_ntff_hook()
    from concourse.bass_utils import run_bass_kernel_spmd

    x = np.asarray(x, dtype=np.float32)
    qw = np.asarray(quantum_weights, dtype=np.float32)
    kmat = np.asarray(kernel, dtype=np.float64)
    bvec = np.asarray(bias, dtype=np.float64)

    U = _build_unitary(qw)
    Ur = U.real
    Ui = U.imag
    # One scale per amplitude row, tied across Re/Im so it squares out of
    # |amp|^2 and divides out of the host-side dense layer.
    rowmax = np.maximum(np.abs(Ur).max(axis=1), np.abs(Ui).max(axis=1))
    s = (_F8_MAX / rowmax).astype(np.float32).astype(np.float64)  # (1024,)
    Urs = np.clip(Ur * s[:, None], -_F8_MAX, _F8_MAX)
    Uis = np.clip(Ui * s[:, None], -_F8_MAX, _F8_MAX)
    # w[t, p, k, j]: j<128 -> Urs[128t+j, 128k+p]; j>=128 -> Uis[128t+j-128, ...]
    Ur4 = Urs.reshape(AT, 128, KT, 128).transpose(0, 2, 3, 1)  # [t, k, p, j]
    Ui4 = Uis.reshape(AT, 128, KT, 128).transpose(0, 2, 3, 1)
    w4 = np.concatenate([Ur4, Ui4], axis=3)                # [AT, KT, 128, 256]
    w4 = np.ascontiguousarray(w4.transpose(0, 2, 1, 3)).astype(_F8)  # [t,p,k,j]

    if "nc" not in _CACHE:
        _CACHE["nc"] = _build_graph()
    nc = _CACHE["nc"]

    in_maps = []
    for c in range(N_CORES):
        xs = x[c * ROWS:(c + 1) * ROWS]                        # [512, 1024]
        # xt[c, p, k', r] = X[r, 128*(2c+k')+p]
        xt = np.ascontiguousarray(
            xs.T.reshape(KT // 2, 2, 128, ROWS).transpose(0, 2, 1, 3)
        ).astype(_F16)
        in_maps.append({"xt": xt, "w": w4})

    res = run_bass_kernel_spmd(nc, in_maps, core_ids=list(range(N_CORES)))
    LAST_RESULTS = res
    # Descale the per-row quantization scales out of the dense layer.
    kd = kmat / (s ** 2)[:, None]                          # (1024, 10) float64
    out = np.empty((BATCH, NUM_OUTPUT), dtype=np.float32)
    for c in range(N_CORES):
        # device emits scaled probs^T blocks: out_d[t, j, r] = s^2*probs[r, 128t+j]
        probs = res.results[c]["out"].astype(np.float64)
        probs = probs.transpose(2, 0, 1).reshape(ROWS, DIM)
        out[c * ROWS:(c + 1) * ROWS] = (probs @ kd + bvec).astype(np.float32)
    return out

